# revision 19
# baseline (speedup 1.0000x reference)
"""GCN message-passing on 8 TRN2 NeuronCores — full forward on device.

Strategy: nodes are sharded across the 8 cores in contiguous ranges balanced
by in-edge count. Each layer runs fully on device:
  - per-edge messages are fetched with dma_gather (int16 indices) from a
    replicated bf16 node-feature table in DRAM (4 sub-tables of <=25600 rows
    so indices fit int16),
  - scatter-add is done on TensorE: for each 128-edge group, a one-hot
    [edge x node-window] matrix (built on DVE from per-edge window-relative
    dst ids, scaled by norm_s[src]*norm_d[dst]) is multiplied against the
    gathered messages, accumulating agg^T in PSUM per 128-node window,
  - agg^T @ W + b, relu, residual on TensorE/ScalarE/VectorE in
    feature-major layout,
  - the updated shard is transposed back to node-major bf16 and exchanged
    with 4 sub-AllGathers to rebuild the replicated table for the next layer.
The embedding matmul is folded into layer 1 (A @ (h W_e) == (A h) W_e), so
layer 1 gathers raw bf16(h0) and no initial exchange is needed.
"""

import math
import numpy as np
import ml_dtypes

import concourse.bacc as bacc
import concourse.bass as bass
import concourse.tile as tile
from concourse import mybir
from concourse.bass_utils import run_bass_kernel_spmd

BF16 = ml_dtypes.bfloat16

N_NODES = 100000
N_EDGES = 1600000
DIM = 128
N_LAYERS = 4
N_CORES = 8

NODE_CAP = 12800          # per-core node capacity (multiple of 512 and 128)
N_SUB = 4                 # sub-tables (int16 index limit)
SUB = NODE_CAP // N_SUB   # 3200 rows per rank per sub-table
TBL = N_CORES * SUB       # 25600 rows per sub-table
W_WIN = 128               # scatter window width (nodes)
CALL_G = 32               # groups per dma_gather call (4096 edges)
W_CHUNK = 512             # free-dim chunk for the weight matmul

F32 = mybir.dt.float32
BF = mybir.dt.bfloat16
I16 = mybir.dt.int16


# ----------------------------------------------------------------------------
# host-side preprocessing
# ----------------------------------------------------------------------------

def preprocess(src, dst, n_nodes, n_cores, node_cap, n_sub, w_win):
    """Shard nodes, bucket/pad edges, build per-core gather/one-hot arrays."""
    E = src.shape[0]
    sub = node_cap // n_sub
    deg_out = np.bincount(src, minlength=n_nodes).astype(np.float32)
    deg_in = np.bincount(dst, minlength=n_nodes).astype(np.float32)
    ns = 1.0 / np.sqrt(np.maximum(deg_out, 1.0))
    nd = 1.0 / np.sqrt(np.maximum(deg_in, 1.0))

    cum = np.cumsum(deg_in)
    bounds = [0]
    for k in range(1, n_cores):
        bounds.append(int(np.searchsorted(cum, k * E / n_cores)))
    bounds.append(n_nodes)
    bounds = np.array(bounds, dtype=np.int64)
    n_real = np.diff(bounds)
    assert n_real.max() <= node_cap, n_real

    rank_of = np.searchsorted(bounds, np.arange(n_nodes), side="right") - 1
    local_of = np.arange(n_nodes) - bounds[rank_of]
    node_k = local_of // sub
    node_row = rank_of * sub + (local_of - node_k * sub)

    # padding edges must gather a row holding real (finite) data
    pad_row = np.zeros(n_sub, dtype=np.int64)
    for k in range(n_sub):
        cand = np.nonzero(node_k == k)[0]
        if cand.size:
            pad_row[k] = node_row[cand[0]]
        # else: bucket k has no real nodes -> no edges -> no groups emitted

    nw = node_cap // w_win
    edge_rank = rank_of[dst]
    cores_raw = []
    maxG = np.zeros((n_sub, nw), dtype=np.int64)
    for c in range(n_cores):
        m = edge_rank == c
        es, ed = src[m], dst[m]
        k = node_k[es]
        row = node_row[es]
        w = (ed - bounds[c]) // w_win
        dst_rel = (ed - bounds[c]) - w * w_win
        coef = (ns[es] * nd[ed]).astype(np.float32)
        order = np.lexsort((w, k))
        cores_raw.append((k[order], row[order], w[order],
                          dst_rel[order], coef[order]))
        cnt = np.zeros((n_sub, nw), dtype=np.int64)
        np.add.at(cnt, (k, w), 1)
        maxG = np.maximum(maxG, (cnt + 127) // 128)
    G_cells = maxG                          # [n_sub, nw] groups per cell
    G_cells[0] = np.maximum(G_cells[0], 1)  # bucket 0 must cover every window
    G_tot = int(G_cells.sum())
    E_pad = G_tot * 128

    # static per-group metadata (same for every core)
    g_w = np.zeros(G_tot, dtype=np.int64)
    g_k = np.zeros(G_tot, dtype=np.int64)
    g_first = np.zeros(G_tot, dtype=bool)
    g_last = np.zeros(G_tot, dtype=bool)
    g = 0
    for k in range(n_sub):
        for w in range(nw):
            n = G_cells[k, w]
            if n == 0:
                continue
            g_w[g:g + n] = w
            g_k[g:g + n] = k
            g_first[g] = True
            g_last[g + n - 1] = True
            g += n

    # per-core padded edge arrays
    per_core = []
    for c in range(n_cores):
        k_, row_, w_, dr_, cf_ = cores_raw[c]
        cnt = np.zeros((n_sub, nw), dtype=np.int64)
        np.add.at(cnt, (k_, w_), 1)
        idx16 = np.zeros(E_pad, dtype=np.int16)
        drel = np.full(E_pad, -1.0, dtype=np.float32)
        cf = np.zeros(E_pad, dtype=np.float32)
        pos = 0
        start = 0
        for k in range(n_sub):
            for w in range(nw):
                n = int(cnt[k, w])
                cap = int(G_cells[k, w]) * 128
                idx16[pos:pos + cap] = pad_row[k]
                idx16[pos:pos + n] = row_[start:start + n]
                drel[pos:pos + n] = dr_[start:start + n]
                cf[pos:pos + n] = cf_[start:start + n]
                start += n
                pos += cap
        # wrap indices: logical edge i -> [i % 16, i // 16], tiled to 128 parts
        idx_wrapped = np.tile(idx16.reshape(-1, 16).T, (8, 1))  # [128, E_pad/16]
        drel_t = np.ascontiguousarray(
            drel.reshape(-1, 128).T.astype(BF16))               # [128, G_tot]
        cf_t = np.ascontiguousarray(
            cf.reshape(-1, 128).T.astype(BF16))                 # [128, G_tot]
        per_core.append((np.ascontiguousarray(idx_wrapped), drel_t, cf_t))

    # gather calls: per sub-bucket, runs of <= CALL_G groups
    calls = []  # (k, g_start, n_groups)
    g = 0
    for k in range(n_sub):
        nk = int(G_cells[k].sum())
        done = 0
        while done < nk:
            n = min(CALL_G, nk - done)
            calls.append((k, g + done, n))
            done += n
        g += nk

    meta = dict(bounds=bounds, n_real=n_real, node_k=node_k,
                node_row=node_row, G_cells=G_cells, G_tot=G_tot,
                E_pad=E_pad, g_w=g_w, g_k=g_k, g_first=g_first,
                g_last=g_last, calls=calls, nw=nw, ns=ns, nd=nd)
    return meta, per_core


# ----------------------------------------------------------------------------
# device graph
# ----------------------------------------------------------------------------

def build_graph(meta, n_layers, node_cap, n_sub, w_win, n_cores,
                embed_on_device, debug_stage=99):
    sub = node_cap // n_sub
    tblr = n_cores * sub
    nw = meta["nw"]
    G_tot = meta["G_tot"]
    E_pad = meta["E_pad"]
    g_w, g_first, g_last = meta["g_w"], meta["g_first"], meta["g_last"]
    calls = meta["calls"]
    n_wchunks = math.ceil(node_cap / W_CHUNK)

    nc = bacc.Bacc("TRN2", target_bir_lowering=False, debug=False,
                   num_devices=n_cores)

    # inputs
    h0T_d = nc.dram_tensor("h0T", [128, node_cap], F32, kind="ExternalInput")
    tbl0_d = [nc.dram_tensor(f"tbl0_{k}", [tblr, DIM], BF,
                             kind="ExternalInput") for k in range(n_sub)]
    idx_d = nc.dram_tensor("eidx", [128, E_pad // 16], I16,
                           kind="ExternalInput")
    drel_d = nc.dram_tensor("edrel", [128, G_tot], BF, kind="ExternalInput")
    coef_d = nc.dram_tensor("ecoef", [128, G_tot], BF, kind="ExternalInput")
    iota_d = nc.dram_tensor("iota", [128, w_win], BF, kind="ExternalInput")
    ident_d = nc.dram_tensor("ident", [128, 128], F32, kind="ExternalInput")
    W_d = nc.dram_tensor("Wmats", [128, n_layers * 128], F32,
                         kind="ExternalInput")
    We_d = nc.dram_tensor("Wemb", [128, 128], F32, kind="ExternalInput")
    bias_d = nc.dram_tensor("bias", [128, n_layers], F32, kind="ExternalInput")
    out_d = nc.dram_tensor("out", [128, node_cap], F32, kind="ExternalOutput")

    # internal DRAM
    myshard = nc.dram_tensor("myshard", [node_cap, DIM], BF, kind="Internal")
    tblbuf = [nc.dram_tensor(f"tblbuf{k}", [tblr, DIM], BF, kind="Internal",
                             addr_space="Shared") for k in range(n_sub)]

    with tile.TileContext(nc) as tc, \
            tc.tile_pool(name="state", bufs=1) as state:
        # persistent SBUF state
        hT = state.tile([128, node_cap], F32, name="hT", tag="hT")
        aggT = state.tile([128, node_cap], F32, name="aggT", tag="aggT")
        idx_sb = state.tile([128, E_pad // 16], I16, name="idx_sb",
                            tag="idx_sb")
        drel_sb = state.tile([128, G_tot], BF, name="drel_sb", tag="drel_sb")
        coef_sb = state.tile([128, G_tot], BF, name="coef_sb", tag="coef_sb")
        iota_sb = state.tile([128, w_win], BF, name="iota_sb", tag="iota_sb")
        ident_sb = state.tile([128, 128], F32, name="ident_sb", tag="ident_sb")
        W_sb = state.tile([128, n_layers * 128], F32, name="W_sb", tag="W_sb")
        We_sb = state.tile([128, 128], F32, name="We_sb", tag="We_sb")
        bias_sb = state.tile([128, n_layers], F32, name="bias_sb",
                             tag="bias_sb")

        nc.sync.dma_start(out=idx_sb[:], in_=idx_d[:])
        nc.sync.dma_start(out=drel_sb[:], in_=drel_d[:])
        nc.sync.dma_start(out=coef_sb[:], in_=coef_d[:])
        nc.sync.dma_start(out=iota_sb[:], in_=iota_d[:])
        nc.sync.dma_start(out=ident_sb[:], in_=ident_d[:])
        nc.sync.dma_start(out=W_sb[:], in_=W_d[:])
        nc.sync.dma_start(out=We_sb[:], in_=We_d[:])
        nc.sync.dma_start(out=bias_sb[:], in_=bias_d[:])

        with (
            tc.tile_pool(name="mpool", bufs=3) as mpool,
            tc.tile_pool(name="opool", bufs=3) as opool,
            tc.tile_pool(name="zpool", bufs=2) as zpool,
            tc.tile_pool(name="tpool", bufs=4) as tpool,
            tc.tile_pool(name="cellp", bufs=4, space="PSUM") as cellp,
            tc.tile_pool(name="zp", bufs=2, space="PSUM") as zp,
            tc.tile_pool(name="trp", bufs=2, space="PSUM") as trp,
        ):
            # prologue: hT = embedding(h0)^T (or plain h0^T if host embedded)
            if embed_on_device:
                nc.sync.dma_start(out=aggT[:], in_=h0T_d[:])
                for ci in range(n_wchunks):
                    s = slice(ci * W_CHUNK, min((ci + 1) * W_CHUNK, node_cap))
                    ep = zp.tile([128, W_CHUNK], F32, name="ep", tag="zpt")
                    nc.tensor.matmul(out=ep[:, :s.stop - s.start],
                                     lhsT=We_sb[:], rhs=aggT[:, s],
                                     start=True, stop=True)
                    nc.vector.tensor_copy(out=hT[:, s],
                                          in_=ep[:, :s.stop - s.start])
            else:
                nc.sync.dma_start(out=hT[:], in_=h0T_d[:])

            for layer in range(n_layers if debug_stage >= 2 else 0):
                tbls = tbl0_d if layer == 0 else tblbuf
                # ---- gather + one-hot scatter ----
                for (k, g0, ng) in calls:
                    e0 = g0 * 128
                    ne = ng * 128
                    mb = mpool.tile([128, CALL_G, DIM], BF, name="mb")
                    nc.gpsimd.dma_gather(
                        out_ap=mb[:, :ng, :],
                        in_ap=tbls[k][:],
                        idxs_ap=idx_sb[:, e0 // 16:(e0 + ne) // 16],
                        num_idxs=ne,
                        num_idxs_reg=ne,
                        elem_size=DIM,
                        single_packet=False,
                    )
                    if debug_stage == 2:
                        nc.vector.tensor_copy(out=aggT[:, :DIM],
                                              in_=mb[:, 0, :])
                        continue
                    ob = opool.tile([128, CALL_G, w_win], BF, name="ob")
                    dr = drel_sb[:, g0:g0 + ng].unsqueeze(2).to_broadcast(
                        [128, ng, w_win])
                    io = iota_sb[:].unsqueeze(1).to_broadcast(
                        [128, ng, w_win])
                    nc.vector.tensor_tensor(out=ob[:, :ng, :], in0=dr, in1=io,
                                            op=mybir.AluOpType.is_equal)
                    cf = coef_sb[:, g0:g0 + ng].unsqueeze(2).to_broadcast(
                        [128, ng, w_win])
                    nc.vector.tensor_tensor(out=ob[:, :ng, :], in0=ob[:, :ng, :],
                                            in1=cf, op=mybir.AluOpType.mult)
                    if debug_stage == 3:
                        nc.vector.tensor_copy(out=aggT[:, :DIM],
                                              in_=ob[:, 0, :])
                        continue
                    for j in range(ng):
                        g = g0 + j
                        if g_first[g]:
                            cps = cellp.tile([128, w_win], F32, name="cps")
                        nc.tensor.matmul(out=cps[:], lhsT=mb[:, j, :],
                                         rhs=ob[:, j, :],
                                         start=bool(g_first[g]),
                                         stop=bool(g_last[g]))
                        if g_last[g]:
                            w = int(g_w[g])
                            dstc = aggT[:, w * w_win:(w + 1) * w_win]
                            if k == 0:
                                nc.vector.tensor_copy(out=dstc, in_=cps[:])
                            else:
                                nc.vector.tensor_tensor(
                                    out=dstc, in0=dstc, in1=cps[:],
                                    op=mybir.AluOpType.add)
                # ---- weight matmul + relu + residual ----
                if debug_stage == 4:
                    continue
                for ci in range(n_wchunks):
                    s = slice(ci * W_CHUNK, min((ci + 1) * W_CHUNK, node_cap))
                    w_ = s.stop - s.start
                    zpt = zp.tile([128, W_CHUNK], F32, name="zpt")
                    nc.tensor.matmul(
                        out=zpt[:, :w_],
                        lhsT=W_sb[:, layer * 128:(layer + 1) * 128],
                        rhs=aggT[:, s], start=True, stop=True)
                    zb = zpool.tile([128, W_CHUNK], F32, name="zb")
                    nc.scalar.activation(
                        out=zb[:, :w_], in_=zpt[:, :w_],
                        func=mybir.ActivationFunctionType.Relu,
                        bias=bias_sb[:, layer:layer + 1], scale=1.0)
                    nc.vector.tensor_tensor(out=hT[:, s], in0=hT[:, s],
                                            in1=zb[:, :w_],
                                            op=mybir.AluOpType.add)
                # ---- table update + exchange (not needed after last layer) --
                if layer < n_layers - 1:
                    for w in range(nw):
                        s = slice(w * 128, (w + 1) * 128)
                        tp = trp.tile([128, 128], F32, name="tp")
                        nc.tensor.transpose(out=tp[:], in_=hT[:, s],
                                            identity=ident_sb[:])
                        tb = tpool.tile([128, 128], BF, name="tb")
                        nc.vector.tensor_copy(out=tb[:], in_=tp[:])
                        nc.sync.dma_start(out=myshard[s, :], in_=tb[:])
                    for k in range(n_sub):
                        nc.gpsimd.collective_compute(
                            "AllGather",
                            mybir.AluOpType.bypass,
                            replica_groups=[list(range(n_cores))],
                            ins=[myshard[k * sub:(k + 1) * sub, :].opt()],
                            outs=[tblbuf[k][:].opt()],
                        )
            if debug_stage in (2, 3, 4):
                nc.sync.dma_start(out=out_d[:], in_=aggT[:])
            else:
                nc.sync.dma_start(out=out_d[:], in_=hT[:])

    nc.compile()
    return nc


# ----------------------------------------------------------------------------
# host wrapper
# ----------------------------------------------------------------------------

def _build_inputs(h, W_embed, b_embed, Ws, bs, meta, per_core, n_layers,
                  node_cap, n_sub, w_win, n_cores):
    sub = node_cap // n_sub
    tblr = n_cores * sub
    bounds, n_real = meta["bounds"], meta["n_real"]
    node_k, node_row = meta["node_k"], meta["node_row"]

    embed_on_device = not np.asarray(b_embed).any()
    if embed_on_device:
        h_base = np.asarray(h, dtype=np.float32)
        W0 = (np.asarray(W_embed, dtype=np.float64)
              @ np.asarray(Ws[0], dtype=np.float64)).astype(np.float32)
    else:
        h_base = (np.asarray(h, dtype=np.float32) @ np.asarray(W_embed)
                  + np.asarray(b_embed)[None, :]).astype(np.float32)
        W0 = np.asarray(Ws[0], dtype=np.float32)

    Wmats = np.concatenate(
        [W0] + [np.asarray(Ws[i], dtype=np.float32)
                for i in range(1, n_layers)], axis=1)          # [128, L*128]
    bias = np.stack([np.asarray(bs[i], dtype=np.float32)
                     for i in range(n_layers)], axis=1)         # [128, L]

    tbl0 = np.zeros((n_sub, tblr, DIM), dtype=BF16)
    tbl0[node_k, node_row] = h_base.astype(BF16)
    tbl0 = [np.ascontiguousarray(tbl0[k]) for k in range(n_sub)]

    iota = np.tile(np.arange(w_win, dtype=np.float32)[None, :],
                   (128, 1)).astype(BF16)
    ident = np.eye(128, dtype=np.float32)

    in_maps = []
    for c in range(n_cores):
        idx_w, drel_t, cf_t = per_core[c]
        h0T = np.zeros((128, node_cap), dtype=np.float32)
        h0T[:, :n_real[c]] = h_base[bounds[c]:bounds[c + 1]].T
        im = {
            "h0T": h0T,
            "eidx": idx_w, "edrel": drel_t, "ecoef": cf_t,
            "iota": iota, "ident": ident,
            "Wmats": Wmats, "Wemb": np.asarray(W_embed, dtype=np.float32),
            "bias": bias,
        }
        for k in range(n_sub):
            im[f"tbl0_{k}"] = tbl0[k]
        in_maps.append(im)
    return in_maps, embed_on_device


_cache = {}


def _install_ntff_hook():
    """Provide antenv.axon_hooks so run_bass_kernel_spmd(trace=True) can
    capture NTFF profiles through the axon tunnel (the agent image's antenv
    lacks the module)."""
    import sys, types, ctypes, contextlib
    try:
        from antenv.axon_hooks import get_axon_ntff_profile_hook  # noqa
        return True
    except ImportError:
        pass
    so_path = "/opt/axon/libaxon_pjrt.so"
    try:
        lib = ctypes.CDLL(so_path)
    except OSError:
        return False
    if not hasattr(lib, "axon_start_nrt_profile"):
        return False
    lib.axon_start_nrt_profile.argtypes = [ctypes.POINTER(ctypes.c_int64),
                                           ctypes.c_size_t]
    lib.axon_start_nrt_profile.restype = ctypes.c_int64
    lib.axon_stop_nrt_profile.argtypes = [ctypes.c_char_p]
    lib.axon_stop_nrt_profile.restype = ctypes.c_int64

    @contextlib.contextmanager
    def _hook(output_dir, device_ids):
        import jax
        jax.devices()
        if device_ids:
            ids = (ctypes.c_int64 * len(device_ids))(*device_ids)
            rc = lib.axon_start_nrt_profile(ids, len(device_ids))
        else:
            rc = lib.axon_start_nrt_profile(None, 0)
        if rc != 0:
            raise RuntimeError(f"axon_start_nrt_profile rc={rc}")
        try:
            yield
        finally:
            n = lib.axon_stop_nrt_profile(str(output_dir).encode())
            if n <= 0:
                print(f"ntff profile: {n} files written to {output_dir}")

    mod = types.ModuleType("antenv.axon_hooks")
    mod._hook = _hook
    mod.get_axon_ntff_profile_hook = lambda: _hook
    mod.set_axon_ntff_profile_hook = lambda h: None
    import antenv
    antenv.axon_hooks = mod
    sys.modules["antenv.axon_hooks"] = mod
    # zero-egress sandbox: skip the artifact upload inside the trace path
    from concourse import bass_utils as _bu
    _bu.upload_artifacts = lambda tmpdir: tmpdir
    return True


def kernel(h, src, dst, W_embed, b_embed, Ws, bs, _trace=False):
    h = np.asarray(h, dtype=np.float32)
    src = np.asarray(src).astype(np.int64)
    dst = np.asarray(dst).astype(np.int64)

    meta, per_core = preprocess(src, dst, N_NODES, N_CORES, NODE_CAP,
                                N_SUB, W_WIN)
    in_maps, embed_on_device = _build_inputs(
        h, W_embed, b_embed, Ws, bs, meta, per_core, N_LAYERS, NODE_CAP,
        N_SUB, W_WIN, N_CORES)

    key = ("graph", meta["G_tot"], embed_on_device)
    if key not in _cache:
        _cache[key] = build_graph(meta, N_LAYERS, NODE_CAP, N_SUB, W_WIN,
                                  N_CORES, embed_on_device)
    nc = _cache[key]

    tmpdir = None
    if _trace:
        _trace = _install_ntff_hook()
        if _trace:
            import tempfile
            tmpdir = tempfile.mkdtemp(prefix="gcn_trace_")
            kernel._last_trace_dir = tmpdir
    res = run_bass_kernel_spmd(nc, in_maps, list(range(N_CORES)),
                               trace=_trace, tmpdir=tmpdir)
    bounds, n_real = meta["bounds"], meta["n_real"]
    out = np.zeros((N_NODES, DIM), dtype=np.float32)
    for c in range(N_CORES):
        out[bounds[c]:bounds[c + 1]] = res.results[c]["out"][:, :n_real[c]].T
    if _trace:
        kernel._last_exec_time_ns = res.exec_time_ns
    return out


# revision 26
# speedup vs baseline: 1.2625x; 1.2625x over previous
"""GCN message-passing on 8 TRN2 NeuronCores — full forward on device.

Strategy: nodes are sharded across the 8 cores in contiguous ranges balanced
by in-edge count. Each layer runs fully on device:
  - per-edge messages are fetched with dma_gather (int16 indices) from a
    replicated bf16 node-feature table in DRAM (4 sub-tables of <=25600 rows
    so indices fit int16),
  - scatter-add is done on TensorE: for each 128-edge group, a one-hot
    [edge x node-window] matrix (built on DVE from per-edge window-relative
    dst ids, scaled by norm_s[src]*norm_d[dst]) is multiplied against the
    gathered messages, accumulating agg^T in PSUM per 128-node window,
  - agg^T @ W + b, relu, residual on TensorE/ScalarE/VectorE in
    feature-major layout,
  - the updated shard is transposed back to node-major bf16 and exchanged
    with 4 sub-AllGathers to rebuild the replicated table for the next layer.
The embedding matmul is folded into layer 1 (A @ (h W_e) == (A h) W_e), so
layer 1 gathers raw bf16(h0) and no initial exchange is needed.
"""

import math
import numpy as np
import ml_dtypes

import concourse.bacc as bacc
import concourse.bass as bass
import concourse.tile as tile
from concourse import mybir
from concourse.bass_utils import run_bass_kernel_spmd

BF16 = ml_dtypes.bfloat16

N_NODES = 100000
N_EDGES = 1600000
DIM = 128
N_LAYERS = 4
N_CORES = 8

NODE_CAP = 12800          # per-core node capacity (multiple of 512 and 128)
N_SUB = 4                 # sub-tables (int16 index limit)
SUB = NODE_CAP // N_SUB   # 3200 rows per rank per sub-table
TBL = N_CORES * SUB       # 25600 rows per sub-table
W_WIN = 128               # scatter window width (nodes)
CALL_G = 16               # groups per dma_gather call (2048 edges)
N_QUEUES = 4              # SWDGE queues (one Q7 core pair each)
W_CHUNK = 512             # free-dim chunk for the weight matmul

F32 = mybir.dt.float32
BF = mybir.dt.bfloat16
I16 = mybir.dt.int16


# ----------------------------------------------------------------------------
# host-side preprocessing
# ----------------------------------------------------------------------------

def preprocess(src, dst, n_nodes, n_cores, node_cap, n_sub, w_win):
    """Shard nodes, bucket/pad edges, build per-core gather/one-hot arrays."""
    E = src.shape[0]
    sub = node_cap // n_sub
    deg_out = np.bincount(src, minlength=n_nodes).astype(np.float32)
    deg_in = np.bincount(dst, minlength=n_nodes).astype(np.float32)
    ns = 1.0 / np.sqrt(np.maximum(deg_out, 1.0))
    nd = 1.0 / np.sqrt(np.maximum(deg_in, 1.0))

    cum = np.cumsum(deg_in)
    bounds = [0]
    for k in range(1, n_cores):
        bounds.append(int(np.searchsorted(cum, k * E / n_cores)))
    bounds.append(n_nodes)
    bounds = np.array(bounds, dtype=np.int64)
    n_real = np.diff(bounds)
    assert n_real.max() <= node_cap, n_real

    rank_of = np.searchsorted(bounds, np.arange(n_nodes), side="right") - 1
    local_of = np.arange(n_nodes) - bounds[rank_of]
    node_k = local_of // sub
    node_row = rank_of * sub + (local_of - node_k * sub)

    # padding edges must gather a row holding real (finite) data
    pad_row = np.zeros(n_sub, dtype=np.int64)
    for k in range(n_sub):
        cand = np.nonzero(node_k == k)[0]
        if cand.size:
            pad_row[k] = node_row[cand[0]]
        # else: bucket k has no real nodes -> no edges -> no groups emitted

    nw = node_cap // w_win
    edge_rank = rank_of[dst]
    cores_raw = []
    maxG = np.zeros((n_sub, nw), dtype=np.int64)
    for c in range(n_cores):
        m = edge_rank == c
        es, ed = src[m], dst[m]
        k = node_k[es]
        row = node_row[es]
        w = (ed - bounds[c]) // w_win
        dst_rel = (ed - bounds[c]) - w * w_win
        coef = (ns[es] * nd[ed]).astype(np.float32)
        order = np.lexsort((w, k))
        cores_raw.append((k[order], row[order], w[order],
                          dst_rel[order], coef[order]))
        cnt = np.zeros((n_sub, nw), dtype=np.int64)
        np.add.at(cnt, (k, w), 1)
        maxG = np.maximum(maxG, (cnt + 127) // 128)
    G_cells = maxG                          # [n_sub, nw] groups per cell
    G_cells[0] = np.maximum(G_cells[0], 1)  # bucket 0 must cover every window
    G_tot = int(G_cells.sum())
    E_pad = G_tot * 128

    # static per-group metadata (same for every core)
    g_w = np.zeros(G_tot, dtype=np.int64)
    g_k = np.zeros(G_tot, dtype=np.int64)
    g_first = np.zeros(G_tot, dtype=bool)
    g_last = np.zeros(G_tot, dtype=bool)
    g = 0
    for k in range(n_sub):
        for w in range(nw):
            n = G_cells[k, w]
            if n == 0:
                continue
            g_w[g:g + n] = w
            g_k[g:g + n] = k
            g_first[g] = True
            g_last[g + n - 1] = True
            g += n

    # per-core padded edge arrays
    per_core = []
    for c in range(n_cores):
        k_, row_, w_, dr_, cf_ = cores_raw[c]
        cnt = np.zeros((n_sub, nw), dtype=np.int64)
        np.add.at(cnt, (k_, w_), 1)
        idx16 = np.zeros(E_pad, dtype=np.int16)
        drel = np.full(E_pad, -1.0, dtype=np.float32)
        cf = np.zeros(E_pad, dtype=np.float32)
        pos = 0
        start = 0
        for k in range(n_sub):
            for w in range(nw):
                n = int(cnt[k, w])
                cap = int(G_cells[k, w]) * 128
                idx16[pos:pos + cap] = pad_row[k]
                idx16[pos:pos + n] = row_[start:start + n]
                drel[pos:pos + n] = dr_[start:start + n]
                cf[pos:pos + n] = cf_[start:start + n]
                start += n
                pos += cap
        # wrap indices: logical edge i -> [i % 16, i // 16], tiled to 128 parts
        idx_wrapped = np.tile(idx16.reshape(-1, 16).T, (8, 1))  # [128, E_pad/16]
        # precomputed one-hot scatter matrices: position (p, g*128+j) holds
        # coef when the edge at (g, p) targets window-relative dst j
        oh = (drel[:, None] == np.arange(w_win, dtype=np.float32)[None, :])
        oh = (oh * cf[:, None]).astype(BF16)                    # [E_pad, 128]
        oh = np.ascontiguousarray(
            oh.reshape(-1, 128, w_win).transpose(1, 0, 2).reshape(128, -1))
        per_core.append((np.ascontiguousarray(idx_wrapped), oh))

    # gather calls: per sub-bucket, runs of <= CALL_G groups
    calls = []  # (k, g_start, n_groups)
    g = 0
    for k in range(n_sub):
        nk = int(G_cells[k].sum())
        done = 0
        while done < nk:
            n = min(CALL_G, nk - done)
            calls.append((k, g + done, n))
            done += n
        g += nk

    meta = dict(bounds=bounds, n_real=n_real, node_k=node_k,
                node_row=node_row, G_cells=G_cells, G_tot=G_tot,
                E_pad=E_pad, g_w=g_w, g_k=g_k, g_first=g_first,
                g_last=g_last, calls=calls, nw=nw, ns=ns, nd=nd)
    return meta, per_core


# ----------------------------------------------------------------------------
# device graph
# ----------------------------------------------------------------------------

def build_graph(meta, n_layers, node_cap, n_sub, w_win, n_cores,
                embed_on_device, debug_stage=99):
    sub = node_cap // n_sub
    tblr = n_cores * sub
    nw = meta["nw"]
    G_tot = meta["G_tot"]
    E_pad = meta["E_pad"]
    g_w, g_first, g_last = meta["g_w"], meta["g_first"], meta["g_last"]
    calls = meta["calls"]
    n_wchunks = math.ceil(node_cap / W_CHUNK)

    nc = bacc.Bacc("TRN2", target_bir_lowering=False, debug=False,
                   num_devices=n_cores, num_swdge_queues=N_QUEUES)

    # inputs
    h0T_d = nc.dram_tensor("h0T", [128, node_cap], F32, kind="ExternalInput")
    tbl0_d = [nc.dram_tensor(f"tbl0_{k}", [tblr, DIM], BF,
                             kind="ExternalInput") for k in range(n_sub)]
    idx_d = nc.dram_tensor("eidx", [128, E_pad // 16], I16,
                           kind="ExternalInput")
    eoh_d = nc.dram_tensor("eoh", [128, G_tot * w_win], BF,
                           kind="ExternalInput")
    ident_d = nc.dram_tensor("ident", [128, 128], F32, kind="ExternalInput")
    W_d = nc.dram_tensor("Wmats", [128, n_layers * 128], F32,
                         kind="ExternalInput")
    We_d = nc.dram_tensor("Wemb", [128, 128], F32, kind="ExternalInput")
    bias_d = nc.dram_tensor("bias", [128, n_layers], F32, kind="ExternalInput")
    out_d = nc.dram_tensor("out", [128, node_cap], F32, kind="ExternalOutput")

    # internal DRAM
    myshard = nc.dram_tensor("myshard", [node_cap, DIM], BF, kind="Internal")
    tblbuf = [nc.dram_tensor(f"tblbuf{k}", [tblr, DIM], BF, kind="Internal",
                             addr_space="Shared") for k in range(n_sub)]

    with tile.TileContext(nc) as tc, \
            tc.tile_pool(name="state", bufs=1) as state:
        # persistent SBUF state
        hT = state.tile([128, node_cap], F32, name="hT", tag="hT")
        aggT = state.tile([128, node_cap], F32, name="aggT", tag="aggT")
        idx_sb = state.tile([128, E_pad // 16], I16, name="idx_sb",
                            tag="idx_sb")
        ident_sb = state.tile([128, 128], F32, name="ident_sb", tag="ident_sb")
        W_sb = state.tile([128, n_layers * 128], F32, name="W_sb", tag="W_sb")
        We_sb = state.tile([128, 128], F32, name="We_sb", tag="We_sb")
        bias_sb = state.tile([128, n_layers], F32, name="bias_sb",
                             tag="bias_sb")

        nc.sync.dma_start(out=idx_sb[:], in_=idx_d[:])
        nc.sync.dma_start(out=ident_sb[:], in_=ident_d[:])
        nc.sync.dma_start(out=W_sb[:], in_=W_d[:])
        nc.sync.dma_start(out=We_sb[:], in_=We_d[:])
        nc.sync.dma_start(out=bias_sb[:], in_=bias_d[:])

        with (
            tc.tile_pool(name="mpool", bufs=3) as mpool,
            tc.tile_pool(name="opool", bufs=3) as opool,
            tc.tile_pool(name="zpool", bufs=2) as zpool,
            tc.tile_pool(name="tpool", bufs=4) as tpool,
            tc.tile_pool(name="cellp", bufs=4, space="PSUM") as cellp,
            tc.tile_pool(name="zp", bufs=2, space="PSUM") as zp,
            tc.tile_pool(name="trp", bufs=2, space="PSUM") as trp,
        ):
            # prologue: hT = embedding(h0)^T, in place chunk by chunk
            nc.sync.dma_start(out=hT[:], in_=h0T_d[:])
            if embed_on_device:
                for ci in range(n_wchunks):
                    s = slice(ci * W_CHUNK, min((ci + 1) * W_CHUNK, node_cap))
                    ep = zp.tile([128, W_CHUNK], F32, name="ep", tag="zpt")
                    nc.tensor.matmul(out=ep[:, :s.stop - s.start],
                                     lhsT=We_sb[:], rhs=hT[:, s],
                                     start=True, stop=True)
                    nc.vector.tensor_copy(out=hT[:, s],
                                          in_=ep[:, :s.stop - s.start])

            qn = 0
            for layer in range(n_layers):
                tbls = tbl0_d if layer == 0 else tblbuf
                # ---- gather + one-hot scatter ----
                for (k, g0, ng) in calls:
                    e0 = g0 * 128
                    ne = ng * 128
                    mb = mpool.tile([128, CALL_G, DIM], BF, name="mb")
                    nc.gpsimd.dma_gather(
                        out_ap=mb[:, :ng, :],
                        in_ap=tbls[k][:],
                        idxs_ap=idx_sb[:, e0 // 16:(e0 + ne) // 16],
                        num_idxs=ne,
                        num_idxs_reg=ne,
                        elem_size=DIM,
                        single_packet=False,
                        queue_num=qn % N_QUEUES,
                    )
                    qn += 1
                    ob = opool.tile([128, CALL_G * w_win], BF, name="ob")
                    nc.sync.dma_start(
                        out=ob[:, :ng * w_win],
                        in_=eoh_d[:, g0 * w_win:(g0 + ng) * w_win])
                    for j in range(ng):
                        g = g0 + j
                        if g_first[g]:
                            cps = cellp.tile([128, w_win], F32, name="cps")
                        nc.tensor.matmul(out=cps[:], lhsT=mb[:, j, :],
                                         rhs=ob[:, j * w_win:(j + 1) * w_win],
                                         start=bool(g_first[g]),
                                         stop=bool(g_last[g]))
                        if g_last[g]:
                            w = int(g_w[g])
                            dstc = aggT[:, w * w_win:(w + 1) * w_win]
                            if k == 0:
                                nc.vector.tensor_copy(out=dstc, in_=cps[:])
                            else:
                                nc.vector.tensor_tensor(
                                    out=dstc, in0=dstc, in1=cps[:],
                                    op=mybir.AluOpType.add)
                # ---- weight matmul + relu + residual ----
                for ci in range(n_wchunks):
                    s = slice(ci * W_CHUNK, min((ci + 1) * W_CHUNK, node_cap))
                    w_ = s.stop - s.start
                    zpt = zp.tile([128, W_CHUNK], F32, name="zpt")
                    nc.tensor.matmul(
                        out=zpt[:, :w_],
                        lhsT=W_sb[:, layer * 128:(layer + 1) * 128],
                        rhs=aggT[:, s], start=True, stop=True)
                    zb = zpool.tile([128, W_CHUNK], F32, name="zb")
                    nc.scalar.activation(
                        out=zb[:, :w_], in_=zpt[:, :w_],
                        func=mybir.ActivationFunctionType.Relu,
                        bias=bias_sb[:, layer:layer + 1], scale=1.0)
                    nc.vector.tensor_tensor(out=hT[:, s], in0=hT[:, s],
                                            in1=zb[:, :w_],
                                            op=mybir.AluOpType.add)
                # ---- table update + exchange (not needed after last layer) --
                if layer < n_layers - 1:
                    cc_after_w = {}  # window index -> [sub-slice k]
                    for k in range(n_sub):
                        wl = (min((k + 1) * sub, node_cap) + 127) // 128 - 1
                        cc_after_w.setdefault(wl, []).append(k)
                    for w in range(nw):
                        s = slice(w * 128, (w + 1) * 128)
                        tp = trp.tile([128, 128], F32, name="tp")
                        nc.tensor.transpose(out=tp[:], in_=hT[:, s],
                                            identity=ident_sb[:])
                        tb = tpool.tile([128, 128], BF, name="tb")
                        nc.vector.tensor_copy(out=tb[:], in_=tp[:])
                        nc.sync.dma_start(out=myshard[s, :], in_=tb[:])
                        for k in cc_after_w.get(w, []):
                            nc.gpsimd.collective_compute(
                                "AllGather",
                                mybir.AluOpType.bypass,
                                replica_groups=[list(range(n_cores))],
                                ins=[myshard[k * sub:(k + 1) * sub, :].opt()],
                                outs=[tblbuf[k][:].opt()],
                            )
            nc.sync.dma_start(out=out_d[:], in_=hT[:])

    nc.compile()
    return nc


# ----------------------------------------------------------------------------
# host wrapper
# ----------------------------------------------------------------------------

def _build_inputs(h, W_embed, b_embed, Ws, bs, meta, per_core, n_layers,
                  node_cap, n_sub, w_win, n_cores):
    sub = node_cap // n_sub
    tblr = n_cores * sub
    bounds, n_real = meta["bounds"], meta["n_real"]
    node_k, node_row = meta["node_k"], meta["node_row"]

    embed_on_device = not np.asarray(b_embed).any()
    if embed_on_device:
        h_base = np.asarray(h, dtype=np.float32)
        W0 = (np.asarray(W_embed, dtype=np.float64)
              @ np.asarray(Ws[0], dtype=np.float64)).astype(np.float32)
    else:
        h_base = (np.asarray(h, dtype=np.float32) @ np.asarray(W_embed)
                  + np.asarray(b_embed)[None, :]).astype(np.float32)
        W0 = np.asarray(Ws[0], dtype=np.float32)

    Wmats = np.concatenate(
        [W0] + [np.asarray(Ws[i], dtype=np.float32)
                for i in range(1, n_layers)], axis=1)          # [128, L*128]
    bias = np.stack([np.asarray(bs[i], dtype=np.float32)
                     for i in range(n_layers)], axis=1)         # [128, L]

    tbl0 = np.zeros((n_sub, tblr, DIM), dtype=BF16)
    tbl0[node_k, node_row] = h_base.astype(BF16)
    tbl0 = [np.ascontiguousarray(tbl0[k]) for k in range(n_sub)]

    ident = np.eye(128, dtype=np.float32)

    in_maps = []
    for c in range(n_cores):
        idx_w, oh = per_core[c]
        h0T = np.zeros((128, node_cap), dtype=np.float32)
        h0T[:, :n_real[c]] = h_base[bounds[c]:bounds[c + 1]].T
        im = {
            "h0T": h0T,
            "eidx": idx_w, "eoh": oh,
            "ident": ident,
            "Wmats": Wmats, "Wemb": np.asarray(W_embed, dtype=np.float32),
            "bias": bias,
        }
        for k in range(n_sub):
            im[f"tbl0_{k}"] = tbl0[k]
        in_maps.append(im)
    return in_maps, embed_on_device


_cache = {}


def _install_ntff_hook():
    """Provide antenv.axon_hooks so run_bass_kernel_spmd(trace=True) can
    capture NTFF profiles through the axon tunnel (the agent image's antenv
    lacks the module)."""
    import sys, types, ctypes, contextlib
    try:
        from antenv.axon_hooks import get_axon_ntff_profile_hook  # noqa
        return True
    except ImportError:
        pass
    so_path = "/opt/axon/libaxon_pjrt.so"
    try:
        lib = ctypes.CDLL(so_path)
    except OSError:
        return False
    if not hasattr(lib, "axon_start_nrt_profile"):
        return False
    lib.axon_start_nrt_profile.argtypes = [ctypes.POINTER(ctypes.c_int64),
                                           ctypes.c_size_t]
    lib.axon_start_nrt_profile.restype = ctypes.c_int64
    lib.axon_stop_nrt_profile.argtypes = [ctypes.c_char_p]
    lib.axon_stop_nrt_profile.restype = ctypes.c_int64

    @contextlib.contextmanager
    def _hook(output_dir, device_ids):
        import jax
        jax.devices()
        if device_ids:
            ids = (ctypes.c_int64 * len(device_ids))(*device_ids)
            rc = lib.axon_start_nrt_profile(ids, len(device_ids))
        else:
            rc = lib.axon_start_nrt_profile(None, 0)
        if rc != 0:
            raise RuntimeError(f"axon_start_nrt_profile rc={rc}")
        try:
            yield
        finally:
            n = lib.axon_stop_nrt_profile(str(output_dir).encode())
            if n <= 0:
                print(f"ntff profile: {n} files written to {output_dir}")

    mod = types.ModuleType("antenv.axon_hooks")
    mod._hook = _hook
    mod.get_axon_ntff_profile_hook = lambda: _hook
    mod.set_axon_ntff_profile_hook = lambda h: None
    import antenv
    antenv.axon_hooks = mod
    sys.modules["antenv.axon_hooks"] = mod
    # zero-egress sandbox: skip the artifact upload inside the trace path
    from concourse import bass_utils as _bu
    _bu.upload_artifacts = lambda tmpdir: tmpdir
    return True


def kernel(h, src, dst, W_embed, b_embed, Ws, bs, _trace=False):
    h = np.asarray(h, dtype=np.float32)
    src = np.asarray(src).astype(np.int64)
    dst = np.asarray(dst).astype(np.int64)

    meta, per_core = preprocess(src, dst, N_NODES, N_CORES, NODE_CAP,
                                N_SUB, W_WIN)
    in_maps, embed_on_device = _build_inputs(
        h, W_embed, b_embed, Ws, bs, meta, per_core, N_LAYERS, NODE_CAP,
        N_SUB, W_WIN, N_CORES)

    key = ("graph", meta["G_tot"], embed_on_device)
    if key not in _cache:
        _cache[key] = build_graph(meta, N_LAYERS, NODE_CAP, N_SUB, W_WIN,
                                  N_CORES, embed_on_device)
    nc = _cache[key]

    tmpdir = None
    if _trace:
        _trace = _install_ntff_hook()
        if _trace:
            import tempfile
            tmpdir = tempfile.mkdtemp(prefix="gcn_trace_")
            kernel._last_trace_dir = tmpdir
    res = run_bass_kernel_spmd(nc, in_maps, list(range(N_CORES)),
                               trace=_trace, tmpdir=tmpdir)
    bounds, n_real = meta["bounds"], meta["n_real"]
    out = np.zeros((N_NODES, DIM), dtype=np.float32)
    for c in range(N_CORES):
        out[bounds[c]:bounds[c + 1]] = res.results[c]["out"][:, :n_real[c]].T
    if _trace:
        kernel._last_exec_time_ns = res.exec_time_ns
    return out
